# revision 29
# baseline (speedup 1.0000x reference)
"""EpisodicMemory Trainium2 kernel (8-core SPMD, slot-sharded).

Reference computation (see problem): attention read over 65536 memory slots
plus a sequential Hebbian EMA scatter write, outputs
(retrieved [2048,512], attention [2048,65536], energy [2048], new_values [65536,512]).

Sharding: slots (N=65536) sharded 8192/core. Two device launches:
  Launch A: per-core keys^T/qp^T prep (PE transposes, float32r), logits matmul,
            exp row-sums (softmax denominator partials), 32-slot chunk maxes
            (argmax candidates), bulk values->new_values copy + bf16 value cast.
  Host glue: combine row sums -> S, exact fp64 argmax over top device chunks,
            sequential-EMA weights (O(B) int work), per-core scatter routing.
  Launch B: attention output (fp32, exp with -log(S) bias), attention^T @ values
            partial (bf16 matmul), Hebbian delta rows = W^T @ write_value +
            scale * gathered value rows, energy.
  Host post: concat attention slices, sum/normalize retrieved partials,
            overlay the <=512 modified rows per core into new_values.
"""

import numpy as np
import ml_dtypes
from contextlib import ExitStack

import concourse.bass as bass
import concourse.mybir as mybir
import concourse.tile as tile
from concourse import bacc
from concourse.bass_utils import run_bass_kernel_spmd
from concourse.masks import make_identity

F32 = mybir.dt.float32
BF16 = mybir.dt.bfloat16
I16 = mybir.dt.int16
AF = mybir.ActivationFunctionType
ALU = mybir.AluOpType
AX = mybir.AxisListType

N_SLOTS = 65536
DV = 512
DQ = 64
B = 2048
BETA = 8.0
LR = 0.01
NCORES = 8
SL = N_SLOTS // NCORES      # 8192 slots per core
NBT = B // 128              # 16 batch tiles
NCH = SL // 512             # 16 slot chunks of 512
NST = SL // 128             # 64 slot tiles of 128
CH32 = SL // 32             # 256 chunk-32 maxes per core
DMAX = 512                  # padded distinct-slot capacity per core

_CACHE = {}
TRACE = False               # set True (test harness) to profile; fills LAST_EXEC_NS
LAST_EXEC_NS = None


def _run(nc, in_maps):
    global LAST_EXEC_NS
    kw = {}
    if TRACE:
        kw = dict(trace=True, trace_cores=list(range(NCORES)))
    res = run_bass_kernel_spmd(nc, in_maps, core_ids=list(range(NCORES)), **kw)
    if TRACE and res.exec_time_ns is not None:
        LAST_EXEC_NS.append(res.exec_time_ns)
    return res


def _build_launch_a():
    nc = bacc.Bacc("TRN2", target_bir_lowering=False, name="epi_a")
    query = nc.dram_tensor("query", [B, DQ], F32, kind="ExternalInput")
    keys_sl = nc.dram_tensor("keys_sl", [SL, DQ], F32, kind="ExternalInput")
    vals_sl = nc.dram_tensor("vals_sl", [SL, DV], F32, kind="ExternalInput")
    wq = nc.dram_tensor("wq", [DQ, DQ], F32, kind="ExternalInput")
    bq = nc.dram_tensor("bq", [DQ], F32, kind="ExternalInput")
    chunkmax = nc.dram_tensor("chunkmax", [B, CH32], F32, kind="ExternalOutput")
    sumexp = nc.dram_tensor("sumexp", [B], F32, kind="ExternalOutput")
    qpt_bf = nc.dram_tensor("qpt_bf", [DQ, B], BF16, kind="ExternalOutput")
    keyst_bf = nc.dram_tensor("keyst_bf", [DQ, SL], BF16, kind="ExternalOutput")
    newv = nc.dram_tensor("newv", [SL, DV], F32, kind="ExternalOutput")

    with tile.TileContext(nc) as tc, ExitStack() as ctx:
        const = ctx.enter_context(tc.tile_pool(name="const", bufs=1))
        resident = ctx.enter_context(tc.tile_pool(name="res", bufs=1))
        tin = ctx.enter_context(tc.tile_pool(name="tin", bufs=8))
        pst_pool = ctx.enter_context(tc.tile_pool(name="pst", bufs=3, space="PSUM"))
        psum_pool = ctx.enter_context(tc.tile_pool(name="psl", bufs=3, space="PSUM"))
        epool = ctx.enter_context(tc.tile_pool(name="esc", bufs=6))
        cmpool = ctx.enter_context(tc.tile_pool(name="cm", bufs=3))
        vcp = ctx.enter_context(tc.tile_pool(name="vcp", bufs=8))

        ident = const.tile([128, 128], F32)
        make_identity(nc, ident[:])
        wq_t = const.tile([DQ, DQ], F32)
        nc.sync.dma_start(wq_t[:], wq[:, :])
        bq_t = const.tile([DQ, 1], F32)
        nc.sync.dma_start(bq_t[:], bq[:].rearrange("(d o) -> d o", o=1))

        # query^T (fp32) via PE transposes
        qT = resident.tile([DQ, B], F32)
        for i in range(NBT):
            qt = tin.tile([128, DQ], F32)
            nc.sync.dma_start(qt[:], query[i * 128:(i + 1) * 128, :])
            ps = pst_pool.tile([DQ, 128], F32)
            nc.tensor.transpose(ps[:], qt[:], ident[:])
            nc.vector.tensor_copy(qT[:, i * 128:(i + 1) * 128], ps[:])

        # qp^T = W_q^T-contract + bias, output bf16
        qpt_sb = resident.tile([DQ, B], BF16)
        for j in range(B // 512):
            ps = psum_pool.tile([DQ, 512], F32)
            nc.tensor.matmul(ps[:], wq_t[:], qT[:, j * 512:(j + 1) * 512])
            nc.scalar.activation(qpt_sb[:, j * 512:(j + 1) * 512], ps[:],
                                 AF.Identity, bias=bq_t[:, 0:1], scale=1.0)
        nc.sync.dma_start(qpt_bf[:, :], qpt_sb[:])

        # keys^T (bf16) via PE transposes
        kt_sb = resident.tile([DQ, SL], BF16)
        for i in range(NST):
            kt = tin.tile([128, DQ], F32)
            nc.sync.dma_start(kt[:], keys_sl[i * 128:(i + 1) * 128, :])
            ps = pst_pool.tile([DQ, 128], F32)
            nc.tensor.transpose(ps[:], kt[:], ident[:])
            nc.vector.tensor_copy(kt_sb[:, i * 128:(i + 1) * 128], ps[:])
        nc.sync.dma_start(keyst_bf[:, :], kt_sb[:])

        # main stats loop: logits chunks -> exp row-sums + chunk-32 maxes
        se_all = resident.tile([128, NBT], F32)
        for bt in range(NBT):
            cmax = cmpool.tile([128, CH32], F32)
            sacc = cmpool.tile([128, NCH], F32, tag="sacc")
            for ch in range(NCH):
                ps = psum_pool.tile([128, 512], F32)
                nc.tensor.matmul(ps[:], qpt_sb[:, bt * 128:(bt + 1) * 128],
                                 kt_sb[:, ch * 512:(ch + 1) * 512])
                esc = epool.tile([128, 512], BF16)
                nc.scalar.activation(esc[:], ps[:], AF.Exp, scale=BETA,
                                     accum_out=sacc[:, ch:ch + 1])
                nc.vector.tensor_reduce(
                    cmax[:, ch * 16:(ch + 1) * 16],
                    ps[:].rearrange("p (g x) -> p g x", x=32),
                    axis=AX.X, op=ALU.max)
            nc.vector.reduce_sum(se_all[:, bt:bt + 1], sacc[:], axis=AX.X)
            nc.sync.dma_start(chunkmax[bt * 128:(bt + 1) * 128, :], cmax[:])
        nc.sync.dma_start(sumexp[:].rearrange("(t p) -> p t", p=128), se_all[:])

        # bulk copy values -> new_values (sparse rows overlaid later on host)
        for i in range(NST):
            vt = vcp.tile([128, DV], F32)
            nc.sync.dma_start(vt[:], vals_sl[i * 128:(i + 1) * 128, :])
            nc.sync.dma_start(newv[i * 128:(i + 1) * 128, :], vt[:])

    nc.finalize()
    return nc


def _build_launch_b():
    nc = bacc.Bacc("TRN2", target_bir_lowering=False, name="epi_b")
    qpt_bf = nc.dram_tensor("qpt_bf", [DQ, B], BF16, kind="ExternalInput")
    keyst_bf = nc.dram_tensor("keyst_bf", [DQ, SL], BF16, kind="ExternalInput")
    vals_sl = nc.dram_tensor("vals_sl", [SL, DV], F32, kind="ExternalInput")
    wv = nc.dram_tensor("wv", [B, DV], F32, kind="ExternalInput")
    wct = nc.dram_tensor("wct", [B, DMAX], BF16, kind="ExternalInput")
    nls = nc.dram_tensor("nls", [B], F32, kind="ExternalInput")   # -log(S)
    scale_pk = nc.dram_tensor("scale_pk", [128, DMAX // 128], F32, kind="ExternalInput")
    didx = nc.dram_tensor("didx", [128, 32], I16, kind="ExternalInput")

    att_sl = nc.dram_tensor("att_sl", [B, SL], F32, kind="ExternalOutput")
    retp = nc.dram_tensor("retp", [DV, B], F32, kind="ExternalOutput")
    rows = nc.dram_tensor("rows", [DMAX, DV], F32, kind="ExternalOutput")
    energy = nc.dram_tensor("energy", [B], F32, kind="ExternalOutput")

    NG = DMAX // 128  # 4 gather groups

    with tile.TileContext(nc) as tc, ExitStack() as ctx:
        resident = ctx.enter_context(tc.tile_pool(name="res", bufs=1))
        psa = ctx.enter_context(tc.tile_pool(name="psa", bufs=2, space="PSUM"))
        psr = ctx.enter_context(tc.tile_pool(name="psr", bufs=1, space="PSUM"))
        psl = ctx.enter_context(tc.tile_pool(name="psl", bufs=2, space="PSUM"))
        psl = ctx.enter_context(tc.tile_pool(name="psl", bufs=2, space="PSUM"))
        attu_pool = ctx.enter_context(tc.tile_pool(name="attu", bufs=8))
        retsb_pool = ctx.enter_context(tc.tile_pool(name="retsb", bufs=4))
        attbig_pool = ctx.enter_context(tc.tile_pool(name="attbig", bufs=2))
        p3 = ctx.enter_context(tc.tile_pool(name="p3", bufs=1))

        # resident inputs
        qpt_sb = resident.tile([DQ, B], BF16)
        nc.sync.dma_start(qpt_sb[:], qpt_bf[:, :])
        kt_sb = resident.tile([DQ, SL], BF16)
        nc.sync.dma_start(kt_sb[:], keyst_bf[:, :])
        vals_bf = resident.tile([128, NST, DV], BF16)
        for i in range(NST):
            nc.gpsimd.dma_start(vals_bf[:, i, :], vals_sl[i * 128:(i + 1) * 128, :])
        nls_sb = resident.tile([128, NBT], F32)
        nc.sync.dma_start(nls_sb[:], nls[:].rearrange("(t p) -> p t", p=128))

        # ---- phases 1+2 interleaved at s-tile granularity ----
        # Each round r: PV pipeline for batch-chunk bc=r (64 s-tiles, PE-heavy)
        # braided with the 64 attention-output chunks of batch-tiles 4r..4r+3
        # (ACT-heavy), so PE and ACT stay co-saturated.
        for r in range(4):
            bc = r
            prs = [psr.tile([128, 512], F32, name=f"psr{v}", tag=f"psr{v}")
                   for v in range(4)]
            abig = None
            for st in range(NST):
                # p1: attU^T tile + 4 PV accumulations
                pa = psa.tile([128, 512], F32, name="pa", tag="pa")
                nc.tensor.matmul(pa[:], kt_sb[:, st * 128:(st + 1) * 128],
                                 qpt_sb[:, bc * 512:(bc + 1) * 512])
                attu = attu_pool.tile([128, 512], BF16, name="attu", tag="attu")
                nc.scalar.activation(attu[:], pa[:], AF.Exp, scale=BETA)
                for vc in range(4):
                    nc.tensor.matmul(prs[vc][:],
                                     vals_bf[:, st, vc * 128:(vc + 1) * 128],
                                     attu[:], start=(st == 0), stop=(st == NST - 1))
                # p2: one normalized-attention chunk
                bt = 4 * r + st // 16
                ch = st % 16
                half, chh = ch // 8, ch % 8
                if chh == 0:
                    abig = attbig_pool.tile([128, SL // 2], F32,
                                            name="abig", tag="abig")
                pl = psl.tile([128, 512], F32, name="pl", tag="pl")
                nc.tensor.matmul(pl[:], qpt_sb[:, bt * 128:(bt + 1) * 128],
                                 kt_sb[:, ch * 512:(ch + 1) * 512])
                nc.scalar.activation(abig[:, chh * 512:(chh + 1) * 512], pl[:],
                                     AF.Exp, scale=BETA, bias=nls_sb[:, bt:bt + 1])
                if chh == 7:
                    nc.sync.dma_start(
                        att_sl[bt * 128:(bt + 1) * 128,
                               half * (SL // 2):(half + 1) * (SL // 2)], abig[:])
            for vc in range(4):
                rsb = retsb_pool.tile([128, 512], F32, name="rsb", tag="rsb")
                nc.vector.tensor_copy(rsb[:], prs[vc][:])
                nc.sync.dma_start(
                    retp[vc * 128:(vc + 1) * 128, bc * 512:(bc + 1) * 512], rsb[:])
        nc.sync.dma_start(energy[:].rearrange("(t p) -> p t", p=128), nls_sb[:])

        # ---- phase 3: Hebbian delta rows ----
        wv_bf = p3.tile([128, NBT, DV], BF16)
        for i in range(NBT):
            nc.gpsimd.dma_start(wv_bf[:, i, :], wv[i * 128:(i + 1) * 128, :])
        wct_sb = p3.tile([128, NBT, DMAX], BF16)
        for i in range(NBT):
            nc.sync.dma_start(wct_sb[:, i, :], wct[i * 128:(i + 1) * 128, :])
        idx_sb = p3.tile([128, 32], I16)
        nc.sync.dma_start(idx_sb[:], didx[:, :])
        scale_sb = p3.tile([128, NG], F32)
        nc.sync.dma_start(scale_sb[:], scale_pk[:, :])

        vrows = p3.tile([128, NG, DV], F32)
        nc.gpsimd.dma_gather(vrows[:], vals_sl[:, :], idx_sb[:],
                             num_idxs=DMAX, num_idxs_reg=DMAX, elem_size=DV)

        rows_sb = p3.tile([128, NG, DV], F32)
        for dt in range(NG):
            pd = psa.tile([128, DV], F32, name="pd", tag="pa")
            for kt in range(NBT):
                nc.tensor.matmul(pd[:], wct_sb[:, kt, dt * 128:(dt + 1) * 128],
                                 wv_bf[:, kt, :], start=(kt == 0), stop=(kt == NBT - 1))
            nc.vector.scalar_tensor_tensor(rows_sb[:, dt, :], vrows[:, dt, :],
                                           scale_sb[:, dt:dt + 1], pd[:],
                                           op0=ALU.mult, op1=ALU.add)
        nc.sync.dma_start(rows[:, :].rearrange("(g p) e -> p g e", p=128), rows_sb[:])

    nc.finalize()
    return nc


def _ema_weights(best_idx):
    """Sequential-EMA reformulation: per-item weight and per-slot scale."""
    cnt = {}
    suffix = np.zeros(B, np.int64)
    for i in range(B - 1, -1, -1):
        s = int(best_idx[i])
        suffix[i] = cnt.get(s, 0)
        cnt[s] = int(suffix[i]) + 1
    w = (LR * (1.0 - LR) ** suffix).astype(np.float64)
    return w, cnt


def kernel(query, write_value, keys, values, W_q, b_q):
    query = np.ascontiguousarray(np.asarray(query, np.float32))
    write_value = np.ascontiguousarray(np.asarray(write_value, np.float32))
    keys = np.ascontiguousarray(np.asarray(keys, np.float32))
    values = np.ascontiguousarray(np.asarray(values, np.float32))
    W_q = np.ascontiguousarray(np.asarray(W_q, np.float32))
    b_q = np.ascontiguousarray(np.asarray(b_q, np.float32))

    global LAST_EXEC_NS
    LAST_EXEC_NS = []

    if "a" not in _CACHE:
        _CACHE["a"] = _build_launch_a()
    nca = _CACHE["a"]

    in_maps_a = []
    for c in range(NCORES):
        in_maps_a.append({
            "query": query,
            "keys_sl": np.ascontiguousarray(keys[c * SL:(c + 1) * SL]),
            "vals_sl": np.ascontiguousarray(values[c * SL:(c + 1) * SL]),
            "wq": W_q,
            "bq": b_q,
        })
    res_a = _run(nca, in_maps_a)
    ra = res_a.results

    # ---- host glue ----
    S = np.zeros(B, np.float64)
    for c in range(NCORES):
        S += np.asarray(ra[c]["sumexp"], np.float64)
    lnS = np.log(S)
    nls = (-lnS).astype(np.float32)

    # global argmax: top device chunk-32 candidates, exact fp64 rescore
    cm = np.concatenate([ra[c]["chunkmax"] for c in range(NCORES)], axis=1)  # [B, 2048]
    TOPK = 8
    top = np.argpartition(-cm, TOPK, axis=1)[:, :TOPK]                 # [B, K]
    cand = (top[:, :, None] * 32 + np.arange(32)[None, None, :]).reshape(B, -1)
    qp64 = query.astype(np.float64) @ W_q.astype(np.float64) + b_q.astype(np.float64)
    ck = keys.astype(np.float64)[cand]                                  # [B, K*32, 64]
    scores = np.einsum("bkd,bd->bk", ck, qp64)
    best_idx = cand[np.arange(B), np.argmax(scores, axis=1)]

    w, cnt = _ema_weights(best_idx)

    if "b" not in _CACHE:
        _CACHE["b"] = _build_launch_b()
    ncb = _CACHE["b"]

    core_of = best_idx // SL
    loc = best_idx - core_of * SL
    host_fallback = False
    in_maps_b = []
    dsl_per_core = []
    for c in range(NCORES):
        sel = np.where(core_of == c)[0]
        dsl = np.unique(loc[sel]) if len(sel) else np.zeros(0, np.int64)
        if len(dsl) > DMAX:
            host_fallback = True
            dsl = dsl[:0]
            sel = sel[:0]
        dsl_per_core.append(dsl)
        wct = np.zeros((B, DMAX), np.float32)
        if len(sel):
            col = np.searchsorted(dsl, loc[sel])
            wct[sel, col] = w[sel].astype(np.float32)
        scale_full = np.ones(DMAX, np.float32)
        for j, s in enumerate(dsl):
            scale_full[j] = (1.0 - LR) ** cnt[int(s + c * SL)]
        full_idx = np.zeros(DMAX, np.int16)
        full_idx[:len(dsl)] = dsl.astype(np.int16)
        didx = np.tile(full_idx.reshape(32, 16).T, (8, 1)).astype(np.int16)
        in_maps_b.append({
            "qpt_bf": np.ascontiguousarray(ra[c]["qpt_bf"]),
            "keyst_bf": np.ascontiguousarray(ra[c]["keyst_bf"]),
            "vals_sl": np.ascontiguousarray(values[c * SL:(c + 1) * SL]),
            "wv": write_value,
            "wct": np.ascontiguousarray(wct.astype(ml_dtypes.bfloat16)),
            "nls": nls,
            "scale_pk": np.ascontiguousarray(scale_full.reshape(DMAX // 128, 128).T),
            "didx": didx,
        })
    res_b = _run(ncb, in_maps_b)
    rb = res_b.results

    # ---- host post ----
    attention = np.concatenate([rb[c]["att_sl"] for c in range(NCORES)], axis=1)
    retp = np.zeros((DV, B), np.float64)
    for c in range(NCORES):
        retp += np.asarray(rb[c]["retp"], np.float64)
    retrieved = (retp.T / S[:, None]).astype(np.float32)
    energy = np.asarray(rb[0]["energy"], np.float32)

    new_values = np.concatenate([ra[c]["newv"] for c in range(NCORES)], axis=0)
    if host_fallback:
        # exceedingly unlikely (needs >512 distinct hit slots in one shard):
        # apply the exact sequential EMA on host
        nv = values.copy()
        for i in range(B):
            s = int(best_idx[i])
            nv[s] = (1.0 - LR) * nv[s] + LR * write_value[i]
        new_values = nv
    else:
        for c in range(NCORES):
            dsl = dsl_per_core[c]
            if len(dsl):
                new_values[c * SL + dsl] = rb[c]["rows"][:len(dsl)]

    return retrieved, attention, energy, new_values


def last_run_exec_times():
    """Stub for test harness: exec times are obtained via trace in test.py."""
    return None
